# revision 5
# baseline (speedup 1.0000x reference)
"""Trainium2 Bass kernel for CodePredictorAttention (B=2, Q=2048, HID=2048,
HQ=16, HKV=4, D=128, causal, qk-rmsnorm + neox rope, GQA).

Sharding (8 cores): data-parallel over batch (2) x tensor-parallel over head
groups (4). Core c handles batch c//4 and q-heads [4g, 4g+4) with kv-head g,
g = c%4. o_proj is row-parallel; the 4 partial outputs per batch are summed
on the host.

Per-core pipeline (all matmuls in float32r: full PE speed, ~12-bit mantissa):
  1. qkv projection  out[tok, feat] = x^T-tiles.T @ w-tiles   (feat = 4q+k+v)
  2. rms-norm scale via DVE (sumsq + rsqrt Newton), applied during PSUM
     eviction (ACT copy with per-partition scale); neox rope on DVE;
     q/k transposed to [D, tok] via PE transposes.
  3. attention in S^T layout: S^T[k,q] = kT.T @ qT (+ causal mask tiles via
     identity matmul), E = exp(S^T * scale) on ACT, O^T[D,q] = V.T @ E and
     colsums = ones.T @ E accumulated on PE; normalize O^T = O^T * (1/sums)
     on DVE.
  4. o_proj out[tok, hid] = O^T-tiles.T @ wo^T-tiles, DMA to DRAM.
"""
import os
import numpy as np
from contextlib import ExitStack

import concourse.bass as bass
import concourse.tile as tile
from concourse import bacc, mybir
from concourse.bass_utils import run_bass_kernel_spmd

B, Q, HID = 2, 2048, 2048
HQ, HKV, D = 16, 4, 128
NQH = HQ // HKV          # q heads per core = 4
EPS = 1e-6
THETA = 1000000.0
SCALE = float(D) ** -0.5
MASK_NEG = -30000.0
P = 128
TOK_T = Q // P           # 16 token tiles
KT = HID // P            # 16 hid contraction tiles
QM = 4                   # q-macro tiles of 512
QMW = Q // QM            # 512
F32 = mybir.dt.float32
F32R = mybir.dt.float32r
I32 = mybir.dt.int32
AF = mybir.ActivationFunctionType
OP = mybir.AluOpType

RSQRT_MAGIC = 0x5F3759DF

last_exec_time_ns = None   # set when BASS_TRACE=1


def _emit(ctx, tc, io, apply_qw, apply_kw):
    nc = tc.nc

    const = ctx.enter_context(tc.tile_pool(name="const", bufs=1))
    xpool = ctx.enter_context(tc.tile_pool(name="xp", bufs=2))
    qkvsb = ctx.enter_context(tc.tile_pool(name="qkvsb", bufs=3))
    rsq = ctx.enter_context(tc.tile_pool(name="rsq", bufs=3))
    big = ctx.enter_context(tc.tile_pool(name="big", bufs=1))
    epool = ctx.enter_context(tc.tile_pool(name="ep", bufs=3))
    opool = ctx.enter_context(tc.tile_pool(name="op", bufs=3))
    psum = ctx.enter_context(tc.tile_pool(name="ps", bufs=6, space="PSUM"))
    psum_kv = ctx.enter_context(tc.tile_pool(name="pskv", bufs=2, space="PSUM"))

    # ---- resident constants / weights ----
    w_sb = const.tile([P, KT, 512 + 2 * P], F32R, tag="wbig")  # qkv w [p, kt, f]
    nc.sync.dma_start(w_sb[:], io["wt"][:].rearrange("(kt p) f -> p kt f", p=P))
    cos_sb = const.tile([P, TOK_T, D // 2], F32)
    nc.sync.dma_start(cos_sb[:], io["cos"][:].rearrange("(t p) d -> p t d", p=P))
    sin_sb = const.tile([P, TOK_T, D // 2], F32)
    nc.sync.dma_start(sin_sb[:], io["sin"][:].rearrange("(t p) d -> p t d", p=P))
    mask_sb = const.tile([P, NQH, QMW], F32R)
    nc.sync.dma_start(mask_sb[:], io["masks"][:].rearrange("r p f -> p r f"))
    ident_sb = const.tile([P, P], F32R)
    nc.sync.dma_start(ident_sb[:], io["ident"][:])
    ones_sb = const.tile([P, P], F32R)
    nc.sync.dma_start(ones_sb[:], io["ones"][:])
    if apply_qw:
        wqrep_sb = const.tile([P, NQH * P], F32)
        nc.sync.dma_start(wqrep_sb[:], io["wqrep"][:])
    if apply_kw:
        wkrep_sb = const.tile([P, P], F32)
        nc.sync.dma_start(wkrep_sb[:], io["wkrep"][:])
    magic_sb = const.tile([P, NQH + 1], I32)
    nc.vector.memset(magic_sb[:], RSQRT_MAGIC)

    # ---- resident activations ----
    qT_sb = big.tile([P, NQH, Q], F32R)      # [D, h, tok]
    kT_sb = big.tile([P, Q], F32R)           # [D, tok]
    v_sb = big.tile([P, TOK_T, D], F32R)     # [tok%128, t, D]
    ot_sb = big.tile([P, NQH, Q], F32R)      # normalized attention out^T [D, h, tok]

    def rsqrt_dve(out_ap, in_ap, n):
        """out = in^-1/2 elementwise on DVE: bit-trick seed + 2 Newton steps."""
        y = rsq.tile([P, n], F32, tag="rs_y")
        sh = rsq.tile([P, n], I32, tag="rs_sh")
        nc.vector.tensor_scalar(sh[:], in_ap.bitcast(I32), 1, None,
                                op0=OP.arith_shift_right)
        nc.vector.tensor_sub(y[:].bitcast(I32), magic_sb[:, :n], sh[:])
        for it in range(2):
            a = rsq.tile([P, n], F32, tag="rs_a")
            c = rsq.tile([P, n], F32, tag="rs_c")
            nc.vector.tensor_mul(a[:], y[:], y[:])
            nc.vector.tensor_mul(a[:], a[:], in_ap)
            nc.vector.tensor_scalar(c[:], a[:], -0.5, 1.5, op0=OP.mult, op1=OP.add)
            if it == 0:
                yn = rsq.tile([P, n], F32, tag="rs_y2")
                nc.vector.tensor_mul(yn[:], y[:], c[:])
                y = yn
            else:
                nc.vector.tensor_mul(out_ap, y[:], c[:])

    # ================= phase 1: qkv + norm + rope + transpose =================
    for t in range(TOK_T):
        x_sb = xpool.tile([P, KT, P], F32R, tag="x")
        nc.sync.dma_start(
            x_sb[:],
            io["xt"][:, t * P:(t + 1) * P].rearrange("(kt p) m -> p kt m", p=P))
        qps = psum.tile([P, NQH * P], F32, tag="a")
        kvps = psum_kv.tile([P, 2 * P], F32, tag="kv")
        for k in range(KT):
            nc.tensor.matmul(qps[:], x_sb[:, k, :], w_sb[:, k, 0:NQH * P],
                             start=(k == 0), stop=(k == KT - 1))
        for k in range(KT):
            nc.tensor.matmul(kvps[:], x_sb[:, k, :], w_sb[:, k, NQH * P:],
                             start=(k == 0), stop=(k == KT - 1))

        # v straight to resident store (f32r rounds)
        nc.vector.tensor_copy(v_sb[:, t, :], kvps[:, P:2 * P])

        # mean-square per head via ACT Square (in every table set) with
        # accum_out; scale folds the 1/D so accum = mean(q^2)
        msq = rsq.tile([P, NQH + 1], F32, tag="msq")
        sq_scale = float(D) ** -0.5
        for h in range(NQH):
            scr = rsq.tile([P, P], F32, tag="scr")
            nc.scalar.activation(scr[:], qps[:, h * P:(h + 1) * P], AF.Square,
                                 scale=sq_scale, accum_out=msq[:, h:h + 1])
        scr = rsq.tile([P, P], F32, tag="scr")
        nc.scalar.activation(scr[:], kvps[:, 0:P], AF.Square,
                             scale=sq_scale, accum_out=msq[:, NQH:NQH + 1])
        msqe = rsq.tile([P, NQH + 1], F32, tag="msqe")
        nc.vector.tensor_scalar(msqe[:], msq[:], EPS, None, op0=OP.add)
        rstd = rsq.tile([P, NQH + 1], F32, tag="rstd")
        rsqrt_dve(rstd[:], msqe[:], NQH + 1)

        # evict q/k with per-head rstd scale folded in (ACT Copy, per-part scale)
        q_s = qkvsb.tile([P, NQH * P], F32, tag="qs")
        for h in range(NQH):
            nc.scalar.activation(q_s[:, h * P:(h + 1) * P],
                                 qps[:, h * P:(h + 1) * P],
                                 AF.Copy, scale=rstd[:, h:h + 1])
        k_s = qkvsb.tile([P, P], F32, tag="ks")
        nc.scalar.activation(k_s[:], kvps[:, 0:P], AF.Copy,
                             scale=rstd[:, NQH:NQH + 1])
        if apply_qw:
            nc.vector.tensor_mul(q_s[:], q_s[:], wqrep_sb[:])
        if apply_kw:
            nc.vector.tensor_mul(k_s[:], k_s[:], wkrep_sb[:])

        # neox rope, fused across the 4 q heads via strided views
        d2 = D // 2
        cosb = cos_sb[:, t:t + 1, :].to_broadcast([P, NQH, d2])
        sinb = sin_sb[:, t:t + 1, :].to_broadcast([P, NQH, d2])
        qv = q_s[:].rearrange("p (h d) -> p h d", h=NQH)
        q_n = qkvsb.tile([P, NQH * P], F32R, tag="qn")
        qnv = q_n[:].rearrange("p (h d) -> p h d", h=NQH)
        t1 = qkvsb.tile([P, NQH * d2], F32, tag="t1")
        t2 = qkvsb.tile([P, NQH * d2], F32, tag="t2")
        t1v = t1[:].rearrange("p (h d) -> p h d", h=NQH)
        t2v = t2[:].rearrange("p (h d) -> p h d", h=NQH)
        nc.vector.tensor_mul(t1v, qv[:, :, 0:d2], cosb)
        nc.vector.tensor_mul(t2v, qv[:, :, d2:D], sinb)
        nc.vector.tensor_sub(qnv[:, :, 0:d2], t1v, t2v)
        nc.vector.tensor_mul(t1v, qv[:, :, d2:D], cosb)
        nc.vector.tensor_mul(t2v, qv[:, :, 0:d2], sinb)
        nc.vector.tensor_add(qnv[:, :, d2:D], t1v, t2v)

        k_n = qkvsb.tile([P, P], F32R, tag="kn")
        kt1 = qkvsb.tile([P, d2], F32, tag="kt1")
        kt2 = qkvsb.tile([P, d2], F32, tag="kt2")
        cs = cos_sb[:, t, :]
        sn = sin_sb[:, t, :]
        nc.vector.tensor_mul(kt1[:], k_s[:, 0:d2], cs)
        nc.vector.tensor_mul(kt2[:], k_s[:, d2:D], sn)
        nc.vector.tensor_sub(k_n[:, 0:d2], kt1[:], kt2[:])
        nc.vector.tensor_mul(kt1[:], k_s[:, d2:D], cs)
        nc.vector.tensor_mul(kt2[:], k_s[:, 0:d2], sn)
        nc.vector.tensor_add(k_n[:, d2:D], kt1[:], kt2[:])

        # transpose q heads and k into [D, tok] resident stores
        for h in range(NQH):
            tp = psum.tile([P, P], F32R, tag="a")
            nc.tensor.transpose(tp[:], q_n[:, h * P:(h + 1) * P], ident_sb[:])
            nc.vector.tensor_copy(qT_sb[:, h, t * P:(t + 1) * P], tp[:])
        tp = psum.tile([P, P], F32R, tag="a")
        nc.tensor.transpose(tp[:], k_n[:], ident_sb[:])
        nc.vector.tensor_copy(kT_sb[:, t * P:(t + 1) * P], tp[:])

    # ================= phase 2: causal attention (S^T layout) =================
    for h in range(NQH):
        for j in range(QM):
            nk = 4 * j + 4          # k-tiles in causal span
            ops_ = psum.tile([P, QMW], F32, tag="a")
            sums = psum.tile([P, QMW], F32, tag="a")
            for i in range(nk):
                diag = i >= 4 * j
                sps = psum.tile([P, QMW], F32, tag="a")
                nc.tensor.matmul(sps[:], kT_sb[:, i * P:(i + 1) * P],
                                 qT_sb[:, h, j * QMW:(j + 1) * QMW],
                                 start=True, stop=not diag)
                if diag:
                    nc.tensor.matmul(sps[:], ident_sb[:],
                                     mask_sb[:, i - 4 * j, :],
                                     start=False, stop=True)
                e = epool.tile([P, QMW], F32R, tag="e")
                nc.scalar.activation(e[:], sps[:], AF.Exp, scale=SCALE)
                nc.tensor.matmul(ops_[:], v_sb[:, i, :], e[:],
                                 start=(i == 0), stop=(i == nk - 1))
                nc.tensor.matmul(sums[:], ones_sb[:], e[:],
                                 start=(i == 0), stop=(i == nk - 1))
            rec = epool.tile([P, QMW], F32, tag="rec")
            nc.vector.reciprocal_approx_fast(out=rec[:], in_=sums[:])
            nc.vector.tensor_mul(ot_sb[:, h, j * QMW:(j + 1) * QMW],
                                 ops_[:], rec[:])

    # ================= phase 3: o_proj =================
    # wo timeshares the qkv-weight SBUF slot (same tag, freed after phase 1)
    wo_sb = const.tile([P, NQH, HID], F32R, tag="wbig")
    nc.sync.dma_start(wo_sb[:], io["wot"][:].rearrange("(kf p) o -> p kf o", p=P))
    for t in range(TOK_T):
        for nh in range(NQH):
            pps = psum.tile([P, QMW], F32, tag="a")
            for kf in range(NQH):
                nc.tensor.matmul(pps[:], ot_sb[:, kf, t * P:(t + 1) * P],
                                 wo_sb[:, kf, nh * QMW:(nh + 1) * QMW],
                                 start=(kf == 0), stop=(kf == NQH - 1))
            o_t = opool.tile([P, QMW], F32, tag="oo")
            if nh % 2 == 0:
                nc.vector.tensor_copy(o_t[:], pps[:])
            else:
                nc.scalar.copy(o_t[:], pps[:])
            nc.sync.dma_start(
                io["out"][t * P:(t + 1) * P, nh * QMW:(nh + 1) * QMW], o_t[:])


_cache = {}


def _build(apply_qw, apply_kw):
    key = (apply_qw, apply_kw)
    if key in _cache:
        return _cache[key]
    nc = bacc.Bacc("TRN2", target_bir_lowering=False, debug=False)
    io = {
        "xt": nc.dram_tensor("xt", (HID, Q), F32R, kind="ExternalInput")[:],
        "wt": nc.dram_tensor("wt", (HID, 512 + 2 * P), F32R, kind="ExternalInput")[:],
        "wot": nc.dram_tensor("wot", (NQH * P, HID), F32R, kind="ExternalInput")[:],
        "cos": nc.dram_tensor("cos", (Q, D // 2), F32, kind="ExternalInput")[:],
        "sin": nc.dram_tensor("sin", (Q, D // 2), F32, kind="ExternalInput")[:],
        "masks": nc.dram_tensor("masks", (NQH, P, QMW), F32R, kind="ExternalInput")[:],
        "ident": nc.dram_tensor("ident", (P, P), F32R, kind="ExternalInput")[:],
        "ones": nc.dram_tensor("ones", (P, P), F32R, kind="ExternalInput")[:],
        "out": nc.dram_tensor("out", (Q, HID), F32, kind="ExternalOutput")[:],
    }
    if apply_qw:
        io["wqrep"] = nc.dram_tensor("wqrep", (P, NQH * P), F32,
                                     kind="ExternalInput")[:]
    if apply_kw:
        io["wkrep"] = nc.dram_tensor("wkrep", (P, P), F32,
                                     kind="ExternalInput")[:]
    with tile.TileContext(nc) as tc:
        with ExitStack() as ctx:
            _emit(ctx, tc, io, apply_qw, apply_kw)
    nc.compile()
    _cache[key] = nc
    return nc


def kernel(positions, hidden_states, k_cache, v_cache, wqkv, wo, q_norm_w,
           k_norm_w, seq_len):
    global last_exec_time_ns
    positions = np.asarray(positions)
    hidden_states = np.asarray(hidden_states, dtype=np.float32)
    wqkv = np.asarray(wqkv, dtype=np.float32)
    wo = np.asarray(wo, dtype=np.float32)
    q_norm_w = np.asarray(q_norm_w, dtype=np.float32)
    k_norm_w = np.asarray(k_norm_w, dtype=np.float32)
    if int(np.asarray(seq_len)) != Q:
        raise NotImplementedError("kernel compiled for seq_len == qlen == 2048")

    apply_qw = not np.all(q_norm_w == 1.0)
    apply_kw = not np.all(k_norm_w == 1.0)
    nc = _build(apply_qw, apply_kw)

    # rope tables per batch (mirrors reference fp32 arithmetic)
    inv_freq = 1.0 / (np.float32(THETA) **
                      (np.arange(0, D, 2, dtype=np.float32) / np.float32(D)))
    # causal mask tiles for the 4 diagonal offsets
    p_idx = np.arange(P, dtype=np.int64)[:, None]
    f_idx = np.arange(QMW, dtype=np.int64)[None, :]
    masks = np.zeros((NQH, P, QMW), dtype=np.float32)
    for r in range(NQH):
        masks[r] = np.where(f_idx >= p_idx + r * P, 0.0, MASK_NEG)
    ident = np.eye(P, dtype=np.float32)
    ones = np.ones((P, P), dtype=np.float32)

    in_maps = []
    for c in range(8):
        b, g = c // 4, c % 4
        xt = np.ascontiguousarray(hidden_states[b].T)
        wq = wqkv[512 * g:512 * (g + 1)]
        wk = wqkv[HQ * D + P * g: HQ * D + P * (g + 1)]
        wv = wqkv[HQ * D + HKV * D + P * g: HQ * D + HKV * D + P * (g + 1)]
        wt = np.ascontiguousarray(np.concatenate([wq, wk, wv], axis=0).T)
        wot = np.ascontiguousarray(wo[:, 512 * g:512 * (g + 1)].T)
        freqs = positions[b].astype(np.float32)[:, None] * inv_freq[None, :]
        m = {
            "xt": xt, "wt": wt, "wot": wot,
            "cos": np.cos(freqs).astype(np.float32),
            "sin": np.sin(freqs).astype(np.float32),
            "masks": masks, "ident": ident, "ones": ones,
        }
        if apply_qw:
            m["wqrep"] = np.broadcast_to(
                np.tile(q_norm_w, NQH)[None, :], (P, NQH * P)).copy()
        if apply_kw:
            m["wkrep"] = np.broadcast_to(k_norm_w[None, :], (P, P)).copy()
        in_maps.append(m)

    trace = bool(os.environ.get("BASS_TRACE"))
    res = run_bass_kernel_spmd(nc, in_maps, core_ids=list(range(8)),
                               trace=trace)
    last_exec_time_ns = res.exec_time_ns

    out = np.empty((B, Q, HID), dtype=np.float32)
    for b in range(B):
        acc = res.results[4 * b]["out"].astype(np.float32).copy()
        for g in range(1, 4):
            acc += res.results[4 * b + g]["out"]
        out[b] = acc
    return out
